# revision 14
# baseline (speedup 1.0000x reference)
"""Trainium2 Bass kernel for nn_MoEAttention (RMSNorm + QKV + RoPE + GQA causal
attention + o-proj + post-RMSNorm + random-routing permutation), tensor-parallel
over attention heads across 8 NeuronCores.

Sharding:
  - core g owns q heads 4g..4g+3 and kv head g (w_qkv column shard, 768 cols)
  - w_o column shard (512 output cols per core); residual + post-norm + the
    token permutation are computed column-sharded, gathered on host.
  - pre-RMSNorm + transpose of x is data-parallel over tokens (256 tokens per
    core) followed by an AllGather of x^T.
  - attention outputs are AllGathered (attn^T, 4 MB/core) so every core can
    compute its output-column shard of the o-projection.
  - routing (top-2 of expert_logits + stable argsort) depends only on an input
    tensor; computed on host in numpy, the permutation gather itself runs on
    device via indirect DMA.
"""

import sys

sys.path.insert(0, "/opt/trn_rl_repo")

import numpy as np

import concourse.bacc as bacc
import concourse.bass as bass
import concourse.mybir as mybir
import concourse.tile as tile
from concourse import bass_utils
from concourse.masks import make_identity

F32 = mybir.dt.float32
F32R = mybir.dt.float32r
I32 = mybir.dt.int32
AF = mybir.ActivationFunctionType
ALU = mybir.AluOpType

N_CORES = 8
N = 2048
H = 4096
NH = 32
NKV = 8
HD = 128
NE = 8
TOPK = 2
ROPE_THETA = 10000.0
EPS = 1e-6
SCALE = HD ** -0.5

QH = NH // N_CORES        # 4 q heads per core
COLS = QH * HD            # 512 output cols per core
QKVC = QH * HD + 2 * HD   # 768 w_qkv cols per core
TPC = N // N_CORES        # 256 tokens per core for phase A
TOKCH = 256               # token chunk in QKV matmul phase
NCH = N // TOKCH
KT = H // 128             # 32 contraction tiles
RG = [list(range(N_CORES))]

_CACHE = {}
LAST_EXEC_NS = None


def _build_program():
    nc = bacc.Bacc("TRN2", target_bir_lowering=False, debug=False,
                   num_devices=N_CORES)

    hsA = nc.dram_tensor("hsA", [TPC, H], F32, kind="ExternalInput")
    hsD = nc.dram_tensor("hsD", [N, COLS], F32, kind="ExternalInput")
    wq = nc.dram_tensor("wq", [H, QKVC], F32, kind="ExternalInput")
    wo = nc.dram_tensor("wo", [H, COLS], F32, kind="ExternalInput")
    wpost = nc.dram_tensor("wpost", [128, COLS], F32, kind="ExternalInput")
    cs2 = nc.dram_tensor("cs2", [128, N], F32, kind="ExternalInput")
    sn2 = nc.dram_tensor("sn2", [128, N], F32, kind="ExternalInput")
    gidx = nc.dram_tensor("gidx", [2 * N, 1], I32, kind="ExternalInput")
    perm = nc.dram_tensor("perm", [2 * N, COLS], F32, kind="ExternalOutput")

    with tile.TileContext(nc) as tc:
        with (
            tc.tile_pool(name="const", bufs=1) as constp,
            tc.tile_pool(name="dram", bufs=1, space="DRAM") as dramp,
        ):
            identity = constp.tile([128, 128], F32)
            make_identity(nc, identity[:])
            ones_col0 = constp.tile([128, 1], F32)
            nc.vector.memset(ones_col0[:], 1.0)
            ones_col = constp.tile([128, 1], F32)
            nc.vector.tensor_copy(ones_col[:].bitcast(F32R), ones_col0[:])
            ones_row0 = constp.tile([1, 128], F32)
            nc.vector.memset(ones_row0[:], 1.0)
            ones_row = constp.tile([1, 128], F32)
            nc.vector.tensor_copy(ones_row[:].bitcast(F32R), ones_row0[:])
            eps_col = constp.tile([128, 1], F32)
            nc.vector.memset(eps_col[:], EPS)
            # permutation matrix swapping partition halves: P[i, j] = 1 iff
            # j == (i + 64) % 128; symmetric, so P.T == P.
            swp0 = constp.tile([128, 128], F32)
            nc.gpsimd.memset(swp0[:], 0.0)
            nc.gpsimd.affine_select(out=swp0[:], in_=swp0[:],
                                    compare_op=ALU.not_equal, fill=1.0,
                                    base=-64, channel_multiplier=-1,
                                    pattern=[[1, 128]])
            nc.gpsimd.affine_select(out=swp0[:], in_=swp0[:],
                                    compare_op=ALU.not_equal, fill=1.0,
                                    base=64, channel_multiplier=-1,
                                    pattern=[[1, 128]])
            swp = constp.tile([128, 128], F32)
            nc.vector.tensor_copy(swp[:].bitcast(F32R), swp0[:])

            xt_in = dramp.tile([H, TPC], F32)
            xt_all = dramp.tile([N_CORES, H, TPC], F32, addr_space="Shared")
            qkvT_d = dramp.tile([QKVC, N], F32)
            attn_in = dramp.tile([COLS, N], F32)
            attnT_all = dramp.tile([NH * HD, N], F32, addr_space="Shared")
            ssq_in = dramp.tile([N, 1], F32)
            ssq_out = dramp.tile([N, 1], F32)
            onorm_d = dramp.tile([N, COLS], F32)

            # ---------------- Phase A: pre-RMSNorm + transpose (my tokens) --
            with (
                tc.tile_pool(name="pa", bufs=1) as pa,
                tc.tile_pool(name="paps", bufs=4, space="PSUM") as paps,
            ):
                x_sb = pa.tile([128, 2, H], F32)
                for tt in range(2):
                    nc.sync.dma_start(out=x_sb[:, tt, :],
                                      in_=hsA[tt * 128:(tt + 1) * 128, :])
                for tt in range(2):
                    sq = pa.tile([128, H], F32, tag="sq", bufs=2)
                    ssq = pa.tile([128, 1], F32, tag="ssq", bufs=2)
                    nc.scalar.activation(sq[:], x_sb[:, tt, :], AF.Square,
                                         accum_out=ssq[:])
                    stdv = pa.tile([128, 1], F32, tag="stdv", bufs=2)
                    nc.scalar.activation(stdv[:], ssq[:], AF.Sqrt,
                                         bias=eps_col[:, :1], scale=1.0 / H)
                    rstd = pa.tile([128, 1], F32, tag="rstd", bufs=2)
                    nc.vector.reciprocal(rstd[:], stdv[:])
                    nc.scalar.mul(x_sb[:, tt, :], x_sb[:, tt, :], rstd[:, :1])
                xt_v = xt_in[:, :].rearrange("(hb h) t -> h hb t", h=128)
                for tt in range(2):
                    for hq in range(KT // 4):
                        ps = paps.tile([128, 512], F32, tag="tp")
                        for i in range(4):
                            hb = hq * 4 + i
                            nc.tensor.transpose(
                                ps[:, i * 128:(i + 1) * 128],
                                x_sb[:, tt, hb * 128:(hb + 1) * 128],
                                identity[:])
                        xts = pa.tile([128, 512], F32, tag="xts", bufs=3)
                        if hq % 2 == 0:
                            nc.vector.tensor_copy(xts[:], ps[:])
                        else:
                            nc.scalar.copy(xts[:], ps[:])
                        nc.sync.dma_start(
                            out=xt_v[:, hq * 4:(hq + 1) * 4,
                                     tt * 128:(tt + 1) * 128],
                            in_=xts[:].rearrange("p (hb t) -> p hb t", hb=4))

            nc.gpsimd.collective_compute(
                "AllGather", ALU.bypass, replica_groups=RG,
                ins=[xt_in.opt()], outs=[xt_all.opt()])

            # ---------------- Phase B: qkv^T = wq^T @ x_norm ----------------
            with (
                tc.tile_pool(name="pbw", bufs=1) as pbw,
                tc.tile_pool(name="pb", bufs=2) as pb,
                tc.tile_pool(name="pbps", bufs=8, space="PSUM") as pbps,
            ):
                wq_sb = pbw.tile([128, KT, QKVC], F32)
                wq_v = wq[:, :].rearrange("(k h) c -> h k c", h=128)
                for k in range(KT):
                    nc.sync.dma_start(out=wq_sb[:, k, :].bitcast(F32R), in_=wq_v[:, k, :].bitcast(F32R))
                for c in range(NCH):
                    xt_sb = pb.tile([128, KT, TOKCH], F32, tag="xtc", bufs=2)
                    for k in range(KT):
                        nc.sync.dma_start(
                            out=xt_sb[:, k, :].bitcast(F32R),
                            in_=xt_all[c, k * 128:(k + 1) * 128, :].bitcast(F32R))
                    for m in range(QKVC // 128):
                        ps = pbps.tile([128, TOKCH], F32, tag="qkvps", bufs=8)
                        for k in range(KT):
                            nc.tensor.matmul(
                                ps[:],
                                wq_sb[:, k, m * 128:(m + 1) * 128].bitcast(F32R),
                                xt_sb[:, k, :].bitcast(F32R),
                                start=(k == 0), stop=(k == KT - 1))
                        st = pb.tile([128, TOKCH], F32, tag="qst", bufs=4)
                        if m % 2 == 0:
                            nc.vector.tensor_copy(st[:], ps[:])
                        else:
                            nc.scalar.copy(st[:], ps[:])
                        nc.sync.dma_start(
                            out=qkvT_d[m * 128:(m + 1) * 128,
                                       c * TOKCH:(c + 1) * TOKCH],
                            in_=st[:])

            # ---------------- Phase C: RoPE + GQA causal attention ----------
            with (
                tc.tile_pool(name="pc", bufs=1) as pc,
                tc.tile_pool(name="pcq", bufs=2) as pcq,
                tc.tile_pool(name="pce", bufs=4) as pce,
                tc.tile_pool(name="pcps", bufs=2, space="PSUM") as pcps,
            ):
                cs_sb = pc.tile([128, N], F32)
                nc.sync.dma_start(out=cs_sb[:], in_=cs2[:, :])
                sn_sb = pc.tile([128, N], F32)
                nc.sync.dma_start(out=sn_sb[:], in_=sn2[:, :])
                kT_sb = pc.tile([128, N], F32)
                nc.sync.dma_start(out=kT_sb[:].bitcast(F32R), in_=qkvT_d[QH * 128:(QH + 1) * 128, :].bitcast(F32R))
                vT_sb = pc.tile([128, N], F32)
                nc.sync.dma_start(out=vT_sb[:], in_=qkvT_d[(QH + 1) * 128:(QH + 2) * 128, :])
                v_nat = pc.tile([128, N], F32)
                for tq in range(4):
                    ps = pcps.tile([128, 512], F32, tag="sT", bufs=3)
                    for i in range(4):
                        t = tq * 4 + i
                        nc.tensor.transpose(ps[:, i * 128:(i + 1) * 128],
                                            vT_sb[:, t * 128:(t + 1) * 128],
                                            identity[:])
                    nc.vector.tensor_copy(v_nat[:, tq * 512:(tq + 1) * 512].bitcast(F32R), ps[:])

                def rope_inplace(xT, tmp, who):
                    # out = xT * cos2 + swap_halves(xT) * sn2, chunked by 512
                    for ch in range(4):
                        sl = slice(ch * 512, (ch + 1) * 512)
                        ps = pcps.tile([128, 512], F32, tag="sT", bufs=3,
                                       name=f"rps_{who}_{ch}")
                        nc.tensor.matmul(ps[:], swp[:].bitcast(F32R),
                                         xT[:, sl].bitcast(F32R),
                                         start=True, stop=True)
                        nc.vector.tensor_mul(tmp[:, sl], ps[:], sn_sb[:, sl])
                        nc.vector.tensor_mul(xT[:, sl].bitcast(F32R), xT[:, sl], cs_sb[:, sl])
                        nc.vector.tensor_add(xT[:, sl].bitcast(F32R), xT[:, sl], tmp[:, sl])

                ktmp = pc.tile([128, N], F32)
                rope_inplace(kT_sb, ktmp, "k")

                for h in range(QH):
                    qT = pcq.tile([128, N], F32, tag="qT", bufs=2)
                    nc.sync.dma_start(out=qT[:].bitcast(F32R), in_=qkvT_d[h * 128:(h + 1) * 128, :].bitcast(F32R))
                    qtmp = pcq.tile([128, N], F32, tag="qtmp", bufs=2)
                    rope_inplace(qT, qtmp, f"q{h}")
                    attn_sb = pcq.tile([128, N], F32, tag="attn", bufs=2)
                    for j in range(4):
                        ps_a = pcps.tile([128, 512], F32, tag="pva", bufs=2)
                        ps_d = pcps.tile([1, 512], F32, tag="den", bufs=1)
                        nt = 4 * j + 4
                        for t in range(nt):
                            ps_s = pcps.tile([128, 512], F32, tag="sT", bufs=3)
                            nc.tensor.matmul(
                                ps_s[:],
                                kT_sb[:, t * 128:(t + 1) * 128].bitcast(F32R),
                                qT[:, j * 512:(j + 1) * 512].bitcast(F32R),
                                start=True, stop=True)
                            ex = pce.tile([128, 512], F32, tag="exp", bufs=4)
                            nc.scalar.activation(ex[:].bitcast(F32R), ps_s[:], AF.Exp,
                                                 scale=SCALE)
                            if t >= 4 * j:
                                # keep where q_local - k_local - (128t-512j) >= 0
                                nc.gpsimd.affine_select(
                                    out=ex[:].bitcast(F32R),
                                    in_=ex[:].bitcast(F32R),
                                    compare_op=ALU.is_ge, fill=0.0,
                                    base=512 * j - 128 * t,
                                    channel_multiplier=-1,
                                    pattern=[[1, 512]])
                            nc.tensor.matmul(
                                ps_a[:],
                                v_nat[:, t * 128:(t + 1) * 128].bitcast(F32R),
                                ex[:].bitcast(F32R),
                                start=(t == 0), stop=(t == nt - 1))
                            nc.tensor.matmul(
                                ps_d[:], ones_col[:].bitcast(F32R),
                                ex[:].bitcast(F32R),
                                start=(t == 0), stop=(t == nt - 1))
                        rc = pcq.tile([1, 512], F32, tag="rc", bufs=2)
                        nc.vector.reciprocal_approx_fast(out=rc[:], in_=ps_d[:])
                        rc2 = pcq.tile([1, 512], F32, tag="rc2", bufs=2)
                        nc.vector.tensor_copy(rc2[:].bitcast(F32R), rc[:])
                        ps_b = pcps.tile([128, 512], F32, tag="bc", bufs=2)
                        nc.tensor.matmul(ps_b[:], ones_row[:].bitcast(F32R),
                                         rc2[:].bitcast(F32R),
                                         start=True, stop=True)
                        rb_sb = pcq.tile([128, 512], F32, tag="rb", bufs=2)
                        nc.scalar.copy(rb_sb[:], ps_b[:])
                        nc.vector.tensor_mul(attn_sb[:, j * 512:(j + 1) * 512],
                                             ps_a[:], rb_sb[:])
                    nc.sync.dma_start(out=attn_in[h * 128:(h + 1) * 128, :],
                                      in_=attn_sb[:])

            nc.gpsimd.collective_compute(
                "AllGather", ALU.bypass, replica_groups=RG,
                ins=[attn_in.opt()], outs=[attnT_all.opt()])

            # ---------------- Phase D: o-proj + residual + post-norm + gather
            with (
                tc.tile_pool(name="pdw", bufs=1) as pdw,
                tc.tile_pool(name="pd", bufs=2) as pd,
                tc.tile_pool(name="pdps", bufs=8, space="PSUM") as pdps,
            ):
                wo_sb = pdw.tile([128, KT, COLS], F32)
                wo_v = wo[:, :].rearrange("(k h) c -> h k c", h=128)
                for k in range(KT):
                    nc.sync.dma_start(out=wo_sb[:, k, :].bitcast(F32R), in_=wo_v[:, k, :].bitcast(F32R))
                wpost_sb = pdw.tile([128, COLS], F32)
                nc.sync.dma_start(out=wpost_sb[:], in_=wpost[:, :])
                s_all = pdw.tile([128, N // 128, COLS], F32)

                for mg in range(2):
                    pso = [pdps.tile([128, COLS], F32, tag="o", bufs=8,
                                     name=f"pso_{mg}_{i}")
                           for i in range(8)]
                    for k in range(KT):
                        at = pd.tile([128, 1024], F32, tag="at", bufs=3)
                        nc.sync.dma_start(
                            out=at[:].bitcast(F32R),
                            in_=attnT_all[k * 128:(k + 1) * 128,
                                          mg * 1024:(mg + 1) * 1024].bitcast(F32R))
                        for mm in range(8):
                            nc.tensor.matmul(
                                pso[mm][:],
                                at[:, mm * 128:(mm + 1) * 128].bitcast(F32R),
                                wo_sb[:, k, :].bitcast(F32R),
                                start=(k == 0), stop=(k == KT - 1))
                    for mm in range(8):
                        m = mg * 8 + mm
                        hs_t = pd.tile([128, COLS], F32, tag="hst", bufs=2)
                        nc.sync.dma_start(out=hs_t[:],
                                          in_=hsD[m * 128:(m + 1) * 128, :])
                        nc.vector.tensor_add(s_all[:, m, :], pso[mm][:], hs_t[:])
                        sqs = pd.tile([128, COLS], F32, tag="sqs", bufs=2)
                        ssqp = pd.tile([128, 1], F32, tag="ssqp", bufs=2)
                        nc.scalar.activation(sqs[:], s_all[:, m, :], AF.Square,
                                             accum_out=ssqp[:])
                        nc.sync.dma_start(out=ssq_in[m * 128:(m + 1) * 128, :],
                                          in_=ssqp[:])

                nc.gpsimd.collective_compute(
                    "AllReduce", ALU.add, replica_groups=RG,
                    ins=[ssq_in.opt()], outs=[ssq_out.opt()])

                rsr = pd.tile([128, N // 128], F32, tag="rsr")
                nc.sync.dma_start(
                    out=rsr[:],
                    in_=ssq_out[:, :].rearrange("(t p) o -> p (t o)", p=128))
                stda = pd.tile([128, N // 128], F32, tag="stda")
                nc.scalar.activation(stda[:], rsr[:], AF.Sqrt,
                                     bias=eps_col[:, :1], scale=1.0 / H)
                rstda = pd.tile([128, N // 128], F32, tag="rstda")
                nc.vector.reciprocal(rstda[:], stda[:])

                for m in range(N // 128):
                    on_t = pd.tile([128, COLS], F32, tag="on", bufs=3)
                    nc.scalar.mul(on_t[:], s_all[:, m, :], rstda[:, m:m + 1])
                    nc.vector.tensor_mul(on_t[:], on_t[:], wpost_sb[:])
                    nc.sync.dma_start(out=onorm_d[m * 128:(m + 1) * 128, :],
                                      in_=on_t[:])

                for r in range(2 * N // 128):
                    gi = pd.tile([128, 1], I32, tag="gi", bufs=4)
                    nc.sync.dma_start(out=gi[:],
                                      in_=gidx[r * 128:(r + 1) * 128, :])
                    g_sb = pd.tile([128, COLS], F32, tag="g", bufs=4)
                    nc.gpsimd.indirect_dma_start(
                        out=g_sb[:], out_offset=None,
                        in_=onorm_d[:, :],
                        in_offset=bass.IndirectOffsetOnAxis(ap=gi[:, :1], axis=0))
                    nc.sync.dma_start(out=perm[r * 128:(r + 1) * 128, :],
                                      in_=g_sb[:])

    nc.compile()
    return nc


def _routing_host(expert_logits):
    el = np.asarray(expert_logits, dtype=np.float32)
    order = np.argsort(-el, axis=-1, kind="stable")
    topk_ids = order[:, :TOPK].astype(np.int32)
    topk_vals = np.take_along_axis(el, topk_ids, axis=-1).astype(np.float32)
    topk_weights = topk_vals / topk_vals.sum(axis=-1, keepdims=True, dtype=np.float32)
    reorder_ids = np.argsort(topk_ids.reshape(-1), kind="stable").astype(np.int32)
    src_rows = (reorder_ids // TOPK).astype(np.int32)
    return topk_weights.astype(np.float32), topk_ids, reorder_ids, src_rows


def _rope_tables(positions):
    pos = np.asarray(positions).astype(np.float32)
    inv_freq = 1.0 / (ROPE_THETA ** (np.arange(0, HD, 2, dtype=np.float32) / HD))
    ang = pos[:, None] * inv_freq[None, :]          # [N, 64]
    cosT = np.cos(ang).T.astype(np.float32)         # [64, N]
    sinT = np.sin(ang).T.astype(np.float32)
    cs2 = np.concatenate([cosT, cosT], axis=0)      # [128, N]
    sn2 = np.concatenate([-sinT, sinT], axis=0)     # [128, N]
    return np.ascontiguousarray(cs2), np.ascontiguousarray(sn2)


def kernel(positions, hidden_states, w_qkv, w_o, w_gate, rms_w_pre, rms_w_post,
           expert_logits):
    global LAST_EXEC_NS
    hs = np.ascontiguousarray(np.asarray(hidden_states, dtype=np.float32))
    w_qkv = np.asarray(w_qkv, dtype=np.float32)
    w_o = np.asarray(w_o, dtype=np.float32)
    w_pre = np.asarray(rms_w_pre, dtype=np.float32)
    w_post = np.asarray(rms_w_post, dtype=np.float32)

    topk_weights, topk_ids, reorder_ids, src_rows = _routing_host(expert_logits)
    cs2, sn2 = _rope_tables(positions)

    wq_scaled = w_pre[:, None] * w_qkv              # fold pre-norm weight

    if "nc" not in _CACHE:
        _CACHE["nc"] = _build_program()
    nc = _CACHE["nc"]

    in_maps = []
    for g in range(N_CORES):
        qcols = wq_scaled[:, g * COLS:(g + 1) * COLS]
        kcol = wq_scaled[:, NH * HD + g * HD: NH * HD + (g + 1) * HD]
        vcol = wq_scaled[:, (NH + NKV) * HD + g * HD: (NH + NKV) * HD + (g + 1) * HD]
        wq_g = np.ascontiguousarray(np.concatenate([qcols, kcol, vcol], axis=1))
        in_maps.append({
            "hsA": np.ascontiguousarray(hs[g * TPC:(g + 1) * TPC, :]),
            "hsD": np.ascontiguousarray(hs[:, g * COLS:(g + 1) * COLS]),
            "wq": wq_g,
            "wo": np.ascontiguousarray(w_o[:, g * COLS:(g + 1) * COLS]),
            "wpost": np.ascontiguousarray(
                np.tile(w_post[g * COLS:(g + 1) * COLS][None, :], (128, 1))),
            "cs2": cs2,
            "sn2": sn2,
            "gidx": np.ascontiguousarray(src_rows[:, None]),
        })

    res = bass_utils.run_bass_kernel_spmd(nc, in_maps,
                                          core_ids=list(range(N_CORES)))
    if res.exec_time_ns is not None:
        LAST_EXEC_NS = res.exec_time_ns

    permuted_output = np.concatenate(
        [res.results[g]["perm"] for g in range(N_CORES)], axis=1)
    return permuted_output, topk_weights, topk_ids, reorder_ids
